# revision 5
# baseline (speedup 1.0000x reference)
"""Trainium2 Bass kernel for nn_DSModelMultiQ (segment_reduce DS rule model).

Math (per sample x):
  literal l: truth_l = op_l(x[feat_l], v_l)   (op: ==, <, >)
  rule r:    active_r = AND of its 4 literals
  z = active @ [logA | logO];  w = exp(z);  q = w[:,10]
  out = [w[:,0:10] - q, q] / clip(sum(w[:,0:10]) - 9 q, 1e-12)

Device pipeline per core (samples transposed: X^T [64, n]):
  PE   : viol^T[slot, s] = one-hot(+/-1) gather of x[feat_slot]  (bit-exact fp32)
  ACT  : s = Sign(viol + bias_slot)   bias = -/+ v  (exact fp32 add, per-partition)
  DVE  : bits = (s == tgt_slot)       tgt in {-1, 0}
  PE   : counts = Seg^T @ bits        (4 slots per rule, rule-major slot order)
  DVE  : active = (counts == 4)
  PE   : z^T += logAO_chunk^T @ active  (bf16 hi+lo split, exact bits)
  PE   : transpose z^T back to [samples, 11]
  ACT  : w = Exp(z)
  DVE  : normalize (sum, a*b+c, clip, recip, fused sub-mul)

Sharding: pure data parallel over samples, 8 cores, identical program,
replicated tables. No collectives.
"""

import os
import numpy as np

# Problem constants (hardcoded per contract)
N_FULL, F, R, LPR, K = 100000, 64, 256, 4, 10
L = R * LPR                      # 1024 literal slots
NCORES = 8
NPC = N_FULL // NCORES           # 12500 samples/core
ST = 512                         # samples per supertile
NST = 25                         # supertiles/core
NPAD = ST * NST                  # 12800 padded samples/core
NCHUNK = L // 128                # 8 slot chunks
EPS = 1e-12

_prog_cache = {}


def _build_program():
    import concourse.bacc as bacc
    import concourse.mybir as mybir
    import concourse.tile as tile

    dt = mybir.dt
    alu = mybir.AluOpType
    act_f = mybir.ActivationFunctionType

    nc = bacc.Bacc("TRN2", target_bir_lowering=False, debug=False)

    xat_d = nc.dram_tensor("xat", [F, NPAD], dt.float32, kind="ExternalInput").ap()
    wall_d = nc.dram_tensor("wall", [F, L], dt.float32, kind="ExternalInput").ap()
    biasall_d = nc.dram_tensor("biasall", [128, NCHUNK], dt.float32, kind="ExternalInput").ap()
    tgtall_d = nc.dram_tensor("tgtall", [128, NCHUNK], dt.float32, kind="ExternalInput").ap()
    segt_d = nc.dram_tensor("segt", [128, 4, 128], dt.bfloat16, kind="ExternalInput").ap()
    laohi_d = nc.dram_tensor("laohi", [128, 2, K + 1], dt.bfloat16, kind="ExternalInput").ap()
    laolo_d = nc.dram_tensor("laolo", [128, 2, K + 1], dt.bfloat16, kind="ExternalInput").ap()
    ident_d = nc.dram_tensor("ident", [K + 1, K + 1], dt.float32, kind="ExternalInput").ap()
    out_d = nc.dram_tensor("out", [NPAD, K + 1], dt.float32, kind="ExternalOutput").ap()

    with tile.TileContext(nc) as tc:
        with tc.tile_pool(name="cpool", bufs=1) as cpool, \
             tc.tile_pool(name="wpool", bufs=2) as wpool, \
             tc.tile_pool(name="pspool", bufs=2, space="PSUM") as pspool:

            xat_s = cpool.tile([F, NPAD], dt.float32, name="xat_s")
            nc.sync.dma_start(xat_s[:], xat_d[:])
            wall_s = cpool.tile([F, L], dt.float32, name="wall_s")
            nc.sync.dma_start(wall_s[:], wall_d[:])
            biasall_s = cpool.tile([128, NCHUNK], dt.float32, name="biasall_s")
            nc.sync.dma_start(biasall_s[:], biasall_d[:])
            tgtall_s = cpool.tile([128, NCHUNK], dt.float32, name="tgtall_s")
            nc.sync.dma_start(tgtall_s[:], tgtall_d[:])
            segt_s = cpool.tile([128, 4, 128], dt.bfloat16, name="segt_s")
            nc.sync.dma_start(segt_s[:], segt_d[:])
            laohi_s = cpool.tile([128, 2, K + 1], dt.bfloat16, name="laohi_s")
            nc.sync.dma_start(laohi_s[:], laohi_d[:])
            laolo_s = cpool.tile([128, 2, K + 1], dt.bfloat16, name="laolo_s")
            nc.sync.dma_start(laolo_s[:], laolo_d[:])
            ident_s = cpool.tile([K + 1, K + 1], dt.float32, name="ident_s")
            nc.sync.dma_start(ident_s[:], ident_d[:])

            for st in range(NST):
                s0 = st * ST
                bits = wpool.tile([128, NCHUNK, ST], dt.bfloat16, name="bits", tag="bits", bufs=2)
                for c in range(NCHUNK):
                    viol = pspool.tile([128, ST], dt.float32, name="viol", tag="viol", bufs=2)
                    nc.tensor.matmul(
                        viol[:],
                        wall_s[:, c * 128:(c + 1) * 128],
                        xat_s[:, s0:s0 + ST],
                        start=True, stop=True,
                    )
                    sgn = wpool.tile([128, ST], dt.bfloat16, name="sgn", tag="sgn", bufs=3)
                    nc.scalar.activation(sgn[:], viol[:], act_f.Sign, bias=biasall_s[:, c:c + 1])
                    nc.vector.tensor_scalar(
                        bits[:, c, :], sgn[:], tgtall_s[:, c:c + 1], None, alu.is_equal)

                actives = []
                for g in range(2):
                    cnt = pspool.tile([128, ST], dt.float32, name="cnt", tag="cnt", bufs=2)
                    for cc in range(4):
                        c = 4 * g + cc
                        nc.tensor.matmul(
                            cnt[:], segt_s[:, cc, :], bits[:, c, :],
                            start=(cc == 0), stop=(cc == 3),
                        )
                    act = wpool.tile([128, ST], dt.bfloat16, name=f"act{g}", tag=f"act{g}", bufs=2)
                    nc.vector.tensor_scalar(act[:], cnt[:], 4.0, None, alu.is_equal)
                    actives.append(act)

                zt = pspool.tile([K + 1, ST], dt.float32, name="zt", tag="zt", bufs=2)
                mms = [(laohi_s, 0), (laolo_s, 0), (laohi_s, 1), (laolo_s, 1)]
                for i, (lao, g) in enumerate(mms):
                    nc.tensor.matmul(
                        zt[:], lao[:, g, :], actives[g][:],
                        start=(i == 0), stop=(i == len(mms) - 1),
                    )
                zts = wpool.tile([K + 1, ST], dt.float32, name="zts", tag="zts", bufs=2)
                nc.vector.tensor_copy(zts[:], zt[:])

                for q4 in range(ST // 128):
                    ztp = pspool.tile([128, K + 1], dt.float32, name="ztp", tag="ztp", bufs=2)
                    nc.tensor.transpose(ztp[:], zts[:, q4 * 128:(q4 + 1) * 128], ident_s[:])
                    wex = wpool.tile([128, K + 1], dt.float32, name="wex", tag="wex", bufs=2)
                    nc.scalar.activation(wex[:], ztp[:], act_f.Exp)
                    ssum = wpool.tile([128, 1], dt.float32, name="ssum", tag="ssum", bufs=2)
                    nc.vector.reduce_sum(ssum[:], wex[:, 0:K], axis=mybir.AxisListType.X)
                    tot = wpool.tile([128, 1], dt.float32, name="tot", tag="tot", bufs=2)
                    nc.vector.scalar_tensor_tensor(
                        tot[:], wex[:, K:K + 1], float(-(K - 1)), ssum[:],
                        op0=alu.mult, op1=alu.add)
                    nc.vector.tensor_scalar_max(tot[:], tot[:], EPS)
                    rc = wpool.tile([128, 1], dt.float32, name="rc", tag="rc", bufs=2)
                    nc.vector.reciprocal(rc[:], tot[:])
                    outt = wpool.tile([128, K + 1], dt.float32, name="outt", tag="outt", bufs=3)
                    nc.vector.scalar_tensor_tensor(
                        outt[:, 0:K], wex[:, 0:K], wex[:, K:K + 1],
                        rc[:, 0:1].broadcast_to((128, K)),
                        op0=alu.subtract, op1=alu.mult)
                    nc.vector.tensor_tensor(
                        outt[:, K:K + 1], wex[:, K:K + 1], rc[:], op=alu.mult)
                    nc.sync.dma_start(
                        out_d[s0 + q4 * 128: s0 + (q4 + 1) * 128, :], outt[:])

    nc.compile()
    return nc




def _install_ntff_shim():
    """The image's antenv package lacks axon_hooks; recreate the NTFF
    profile hook via ctypes against libaxon_pjrt.so (profiling only)."""
    import sys, types, ctypes, contextlib

    if "antenv.axon_hooks" in sys.modules:
        return
    try:
        lib = ctypes.CDLL("/opt/axon/libaxon_pjrt.so")
        if not hasattr(lib, "axon_start_nrt_profile"):
            return
    except OSError:
        return
    lib.axon_start_nrt_profile.argtypes = [
        ctypes.POINTER(ctypes.c_int64), ctypes.c_size_t]
    lib.axon_start_nrt_profile.restype = ctypes.c_int64
    lib.axon_stop_nrt_profile.argtypes = [ctypes.c_char_p]
    lib.axon_stop_nrt_profile.restype = ctypes.c_int64

    @contextlib.contextmanager
    def _hook(output_dir, device_ids):
        import jax
        jax.devices()
        if device_ids:
            ids = (ctypes.c_int64 * len(device_ids))(*device_ids)
            rc = lib.axon_start_nrt_profile(ids, len(device_ids))
        else:
            rc = lib.axon_start_nrt_profile(None, 0)
        if rc != 0:
            raise RuntimeError(f"axon_start_nrt_profile rc={rc}")
        try:
            yield
        finally:
            n = lib.axon_stop_nrt_profile(str(output_dir).encode())
            print(f"profile: {n} ntff file(s) written to {output_dir}", file=sys.stderr)

    mod = types.ModuleType("antenv.axon_hooks")
    mod._hook = _hook
    mod.get_axon_ntff_profile_hook = lambda: _hook
    mod.set_axon_ntff_profile_hook = lambda h: None
    sys.modules["antenv.axon_hooks"] = mod

    # avoid remote artifact upload in sandbox
    import concourse.bass_utils as bu
    bu.upload_artifacts = lambda tmpdir: tmpdir


def _softmax64(x):
    x = x.astype(np.float64)
    x = x - x.max(axis=-1, keepdims=True)
    e = np.exp(x)
    return e / e.sum(axis=-1, keepdims=True)


def kernel(X, rule_mass_params, lit_feat_idx, lit_op_code, lit_value, lit2rule, rule_len):
    from concourse.bass_utils import run_bass_kernel_spmd
    import ml_dtypes

    X = np.asarray(X, dtype=np.float32)
    rule_mass_params = np.asarray(rule_mass_params, dtype=np.float32)
    lit_feat_idx = np.asarray(lit_feat_idx, dtype=np.int32)
    lit_op_code = np.asarray(lit_op_code, dtype=np.int32)
    lit_value = np.asarray(lit_value, dtype=np.float32)
    lit2rule = np.asarray(lit2rule, dtype=np.int32)
    rule_len = np.asarray(rule_len, dtype=np.int32)

    n, f = X.shape
    assert (n, f) == (N_FULL, F)
    assert rule_len.shape[0] == R and np.all(rule_len == LPR)

    # --- slot table: literals ordered rule-major (stable sort by rule) ---
    order = np.argsort(lit2rule, kind="stable")
    assert np.all(np.bincount(lit2rule, minlength=R) == LPR)
    feat_o = lit_feat_idx[order]
    op_o = lit_op_code[order]
    val_o = lit_value[order]

    # one-hot gather weights, per-slot bias and sign-target
    wall = np.zeros((F, L), dtype=np.float32)
    bias = np.zeros(L, dtype=np.float32)
    tgt = np.zeros(L, dtype=np.float32)
    for j in range(L):
        fj, oj, vj = int(feat_o[j]), int(op_o[j]), val_o[j]
        if oj == 1:            # x < v  <=> sign(x - v) == -1
            wall[fj, j] = 1.0
            bias[j] = -vj
            tgt[j] = -1.0
        elif oj == 2:          # x > v  <=> sign(v - x) == -1
            wall[fj, j] = -1.0
            bias[j] = vj
            tgt[j] = -1.0
        else:                  # x == v <=> sign(x - v) == 0
            wall[fj, j] = 1.0
            bias[j] = -vj
            tgt[j] = 0.0
    biasall = bias.reshape(NCHUNK, 128).T.copy()          # [128, 8]
    tgtall = tgt.reshape(NCHUNK, 128).T.copy()

    # segment matrix: slot s (within a 128-chunk) -> rule s//4 (within 32)
    segt = np.zeros((128, 4, 128), dtype=ml_dtypes.bfloat16)
    for cc in range(4):
        segt[np.arange(128), cc, 32 * cc + np.arange(128) // 4] = 1.0

    # rule masses -> log tables, rule-permuted is identity here (rules in order)
    m = _softmax64(rule_mass_params)
    logA = np.log(m[:, :K] + m[:, K:K + 1] + EPS)          # [R, K]
    logO = np.log(m[:, K] + EPS)                            # [R]
    lao = np.concatenate([logA, logO[:, None]], axis=1).astype(np.float32)  # [R, 11]
    lao_hi = lao.astype(ml_dtypes.bfloat16)
    lao_lo = (lao - lao_hi.astype(np.float32)).astype(ml_dtypes.bfloat16)
    laohi = lao_hi.reshape(2, 128, K + 1).transpose(1, 0, 2).copy()  # [128, 2, 11]
    laolo = lao_lo.reshape(2, 128, K + 1).transpose(1, 0, 2).copy()

    ident = np.eye(K + 1, dtype=np.float32)

    # --- shard X across cores, transpose, pad ---
    in_maps = []
    for c in range(NCORES):
        xs = X[c * NPC:(c + 1) * NPC]                       # [12500, 64]
        xat = np.zeros((F, NPAD), dtype=np.float32)
        xat[:, :NPC] = xs.T
        in_maps.append(dict(
            xat=xat, wall=wall, biasall=biasall, tgtall=tgtall, segt=segt,
            laohi=laohi, laolo=laolo, ident=ident,
        ))

    if "nc" not in _prog_cache:
        _prog_cache["nc"] = _build_program()
    nc = _prog_cache["nc"]

    trace = bool(int(os.environ.get("BASSK_TRACE", "0")))
    if trace:
        _install_ntff_shim()
    res = run_bass_kernel_spmd(nc, in_maps, list(range(NCORES)), trace=trace)
    if trace and res.exec_time_ns is not None:
        print(f"HW exec time: {res.exec_time_ns} ns")
        _prog_cache["exec_time_ns"] = res.exec_time_ns

    out = np.concatenate(
        [res.results[c]["out"][:NPC] for c in range(NCORES)], axis=0)
    return out.astype(np.float32)
